# revision 4
# baseline (speedup 1.0000x reference)
"""Bilateral filter (7x7, sigma_color=0.1) Trainium2 Bass kernel — Design S2.

Strategy (vs. the strip-layout baseline):
  - Shard H across 8 cores (90 rows each), full 1280 width. Flat row layout:
    SBUF tile IM[102, 3, 1292] fp16 = rows -6..95, channel-major, cols -6..1285.
  - Weight-field symmetry: W_{dy,dx}[u] == W_{6-dy,6-dx}[u + (dy-3,dx-3)], so
    only 24 shift-pairs (+ the free center shift) need D/exp. Per pair j=(ay,ax):
      SB = IM[shifted] - IM[center]            (DVE, [96,3,646] fp16, 2x mode)
      SQ = SB^2                                (ACT Square)
      D  = SQ0+SQ1+SQ2                         (Pool, 2 fused scalar_tensor_tensor)
      F  = exp(-50*D + b_j)                    (ACT, b_j = ln(norm*g_j))
      H  = F * IM[shifted]   (num term j)      (DVE, [90,3,640])
      G  = F * IM[center]    (num term j')     (DVE 2ch + Pool 1ch, [96,3,646])
  - Accumulation on PE into PSUM[90, 4, 640] fp32 (num0..2, den), 2 x-passes
    of 640 cols (PSUM capacity). All matmuls use ONE stationary identity lhsT;
    row/col mirror shifts are rhs partition-offset / column-offset views:
      num += H                      (lhsT=Id, rhs=H)
      num += G[r-ay, x-ax]          (lhsT=Id, rhs=G partitions 3-ay.., cols -ax)
      den += F[r,x] + F[r-ay,x-ax]  (same, rhs=F views)
      center: num += c0*IM, den += c0   (c0 = norm*g_33; tiny extra matmuls)
  - Finalize per pass: rec = 1/den (DVE), out = num*rec (DVE fp32), DMA out.
  - HBM traffic ~2.2 MB/core (vs 38.7 MB for the host-expanded strip design).
"""

import math

import numpy as np

import concourse.bass as bass
import concourse.bacc as bacc
import concourse.mybir as mybir
from concourse.tile import TileContext

F16 = np.float16
F32 = np.float32

H, W, C = 720, 1280, 3
K = 7
PAD = 3
SIGMA_COLOR = 0.1
NORM_COLOR = 1.0 / (2.0 * math.pi * SIGMA_COLOR**2)
EXP_SCALE = -1.0 / (2.0 * SIGMA_COLOR**2)  # -50.0

N_CORES = 8
RPC = H // N_CORES           # 90 output rows per core
XW = 640                     # pass width (PSUM capacity: 4 * 640 fp32 = 5 banks)
NPASS = W // XW              # 2
EXT = 12                     # input halo rows (2*2*PAD)
PW = W + 12                  # padded width 1292

# shift pairs: (ay, ax) with ay<0, or ay==0 and ax<0  (24 of 49; center free)
PAIRS = [(dy - 3, dx - 3) for dy in range(K) for dx in range(K)
         if (dy < 3) or (dy == 3 and dx < 3)]
assert len(PAIRS) == 24

# psum bank-aligned chunks per quantity: (col_start, ncols) within [0, 640),
# global col = 640*q + cs must not cross a 512-col (2KB) bank boundary
CHUNKS = {
    0: [(0, 512), (512, 128)],
    1: [(0, 384), (384, 256)],
    2: [(0, 256), (256, 384)],
    3: [(0, 128), (128, 512)],
}


def _alu(name):
    return getattr(mybir.AluOpType, name)


def build_nc(rows: int = RPC):
    """One core: rows output rows (+12 halo), full width, 2 passes of XW."""
    dt = mybir.dt
    nc = bacc.Bacc("TRN2", debug=False)

    np_ = rows + EXT            # input tile partitions (102)
    fe = rows + 2 * PAD         # F/G extension rows (96)

    IM = nc.dram_tensor("IM", [np_, C, PW], dt.float16, kind="ExternalInput")
    WT = nc.dram_tensor("WT", [fe, 540], dt.float16, kind="ExternalInput")
    BI = nc.dram_tensor("BI", [128, 24], dt.float32, kind="ExternalInput")
    ON = nc.dram_tensor("ON", [1, XW], dt.float16, kind="ExternalInput")
    OUT = nc.dram_tensor("OUT", [C, rows, W], dt.float32, kind="ExternalOutput")

    with TileContext(nc) as tc:
        with (
            tc.tile_pool(name="persist", bufs=1) as pp,
            tc.tile_pool(name="sub", bufs=3) as psub,
            tc.tile_pool(name="sq", bufs=3) as psq,
            tc.tile_pool(name="dd", bufs=4) as pdd,
            tc.tile_pool(name="ff", bufs=4) as pf,
            tc.tile_pool(name="hh", bufs=3) as ph,
            tc.tile_pool(name="gg", bufs=3) as pg,
            tc.tile_pool(name="fin", bufs=2) as pfin,
            tc.tile_pool(name="psum", bufs=1, space="PSUM") as pps,
        ):
            # SBUF compute ops require base partition 0 (or 32/64/96), so
            # row shifts can't be partition-offset views. Instead build 7
            # row-shifted SBUF copies via DMA (no partition restriction):
            # IMS[d][p, c, xi] = Ipad[row p-3+d, col xi],  p=0..95
            IMS = {}
            for d in range(-PAD, PAD + 1):
                t = pp.tile([fe, C, PW], dt.float16, tag=f"ims{d}")
                nc.sync.dma_start(t[:, :, :], IM[3 + d:3 + d + fe, :, :])
                IMS[d] = t
            wt = pp.tile([fe, 540], dt.float16, tag="wt")
            nc.sync.dma_start(wt[:, :], WT[:, :])
            bi = pp.tile([128, 24], dt.float32, tag="bi")
            nc.sync.dma_start(bi[:, :], BI[:, :])
            ones = pp.tile([1, XW], dt.float16, tag="ones")
            nc.sync.dma_start(ones[:, :], ON[:, :])

            # lhsT views, all [fe, rows] base partition 0:
            # SH[k][p, r] = 1 iff p == r + k (k=3..6); CID = c0*SH[3]; OC = c0 row
            SH = {k: wt[:, (k - 3) * rows:(k - 2) * rows] for k in range(3, 7)}
            CID = wt[:, 4 * rows:5 * rows]
            OC = wt[0:1, 5 * rows:6 * rows]

            for p_i in range(NPASS):
                x0 = p_i * XW
                ps = pps.tile([rows, 4, XW], dt.float32, tag="ps", name=f"ps{p_i}")
                started = set()  # PSUM 2KB banks with an open accumulation group

                def mm(q, lhsT, rhs_of, stop_banks=None):
                    """Accumulate psum[:, q, :] += lhsT.T @ rhs chunks.
                    start=True only on the first matmul touching a bank
                    (marks the whole bank pending-zero); stop closes it."""
                    for ci, (cs, cn) in enumerate(CHUNKS[q]):
                        bank = (2560 * q + 4 * cs) // 2048
                        st = bank not in started
                        started.add(bank)
                        sp = stop_banks is not None and bank in stop_banks
                        nc.tensor.matmul(
                            ps[:, q, cs:cs + cn], lhsT, rhs_of(cs, cn),
                            start=st, stop=sp,
                        )

                for jp, (ay, ax) in enumerate(PAIRS):
                    # --- D / F over extension region: partition p = r'+3,
                    #     r' in -3..rows+2; tile col xi = x'-x0+3, x' in -3..642
                    sb = psub.tile([fe, C, 646], dt.float16, tag="sb", name="sb")
                    nc.vector.tensor_tensor(
                        sb[:, :, :],
                        IMS[ay][:, :, x0 + 3 + ax:x0 + 649 + ax],
                        IMS[0][:, :, x0 + 3:x0 + 649],
                        _alu("subtract"),
                    )
                    sq = psq.tile([fe, C, 646], dt.float16, tag="sq", name="sq")
                    nc.scalar.activation(
                        sq[:, :, :], sb[:, :, :],
                        mybir.ActivationFunctionType.Square,
                    )
                    d1 = pdd.tile([fe, 646], dt.float16, tag="d1", name="d1")
                    nc.gpsimd.tensor_tensor(
                        d1[:, :], sq[:, 0, :], sq[:, 1, :], _alu("add"),
                    )
                    dd = pdd.tile([fe, 646], dt.float16, tag="dd", name="dd")
                    nc.vector.tensor_tensor(
                        dd[:, :], sq[:, 2, :], d1[:, :], _alu("add"),
                    )
                    ff = pf.tile([fe, 646], dt.float16, tag="ff", name="ff")
                    nc.scalar.activation(
                        ff[:, :], dd[:, :],
                        mybir.ActivationFunctionType.Exp,
                        bias=bi[0:fe, jp:jp + 1], scale=float(EXP_SCALE),
                    )
                    # --- products, one op per channel (a broadcast operand
                    # drops DVE to 1x rate on hardware; plain views keep 2x)
                    hh = ph.tile([fe, C, XW], dt.float16, tag="hh", name="hh")
                    for c in range(C):
                        nc.vector.tensor_tensor(
                            hh[:, c, :],
                            ff[:, 3:3 + XW],
                            IMS[ay][:, c, x0 + 6 + ax:x0 + 646 + ax],
                            _alu("mult"),
                        )
                    gg = pg.tile([fe, C, 646], dt.float16, tag="gg", name="gg")
                    for c in range(2):
                        nc.vector.tensor_tensor(
                            gg[:, c, :],
                            ff[:, :],
                            IMS[0][:, c, x0 + 3:x0 + 649],
                            _alu("mult"),
                        )
                    nc.gpsimd.tensor_tensor(
                        gg[:, 2, :],
                        IMS[0][:, 2, x0 + 3:x0 + 649], ff[:, :], _alu("mult"),
                    )
                    # --- accumulate: H-term (SH[3]) and G-term (SH[3-ay],
                    # col shift via rhs view)
                    ks = 3 - ay
                    for q in range(C):
                        mm(q, SH[3], lambda cs, cn, q=q: hh[:, q, cs:cs + cn])
                    mm(3, SH[3], lambda cs, cn: ff[:, 3 + cs:3 + cs + cn])
                    for q in range(C):
                        mm(q, SH[ks], lambda cs, cn, q=q: gg[
                            :, q, 3 - ax + cs:3 - ax + cs + cn])
                    mm(3, SH[ks], lambda cs, cn: ff[
                        :, 3 - ax + cs:3 - ax + cs + cn])

                # --- center shift: num += c0*I, den += c0. Last matmul
                # touching each bank closes its accumulation group.
                cen = [(q, cs, cn) for q in range(C) for cs, cn in CHUNKS[q]]
                cen += [(3, cs, cn) for cs, cn in CHUNKS[3]]
                banks = [(2560 * q + 4 * cs) // 2048 for q, cs, cn in cen]
                for i, (q, cs, cn) in enumerate(cen):
                    rhs = (ones[:, cs:cs + cn] if q == 3
                           else IMS[0][:, q, x0 + 6 + cs:x0 + 6 + cs + cn])
                    nc.tensor.matmul(
                        ps[:, q, cs:cs + cn], OC if q == 3 else CID, rhs,
                        start=False, stop=banks[i] not in banks[i + 1:],
                    )

                # --- finalize pass (den is well inside fp32 normal range)
                rec = pfin.tile([rows, XW], dt.float32, tag="rec", name="rec")
                nc.vector.reciprocal_approx_fast(rec[:, :], ps[:, 3, :])
                ot = pfin.tile([rows, C, XW], dt.float32, tag="ot", name="ot")
                nc.vector.tensor_tensor(
                    ot[:, :, :], ps[:, 0:3, :],
                    rec[:, None, :].to_broadcast([rows, C, XW]),
                    _alu("mult"),
                )
                for c in range(C):
                    nc.sync.dma_start(OUT[c, :, x0:x0 + XW], ot[:, c, :])

    nc.compile()
    return nc


def host_prepare(I: np.ndarray, gw49: np.ndarray):
    """I: (1, C, H, W) fp32, gw49: (49,). Returns per-core input maps."""
    _, c_, him, wim = I.shape
    rows = him // N_CORES
    np_ = rows + EXT

    Ip = np.zeros((C, him + EXT, wim + EXT), dtype=F16)
    Ip[:, 6:6 + him, 6:6 + wim] = I[0].astype(F16)

    g7 = gw49.reshape(K, K).astype(np.float64)
    c0 = float(NORM_COLOR * g7[3, 3])

    fe = rows + 2 * PAD
    wt = np.zeros((fe, 540), dtype=F16)
    idx = np.arange(rows)
    for k in range(3, 7):                  # SH[k]: p == r+k
        wt[idx + k, (k - 3) * rows + idx] = 1.0
    wt[idx + 3, 4 * rows + idx] = c0       # CID = c0 * SH[3]
    wt[0, 5 * rows:6 * rows] = c0          # OC row

    bi = np.zeros((128, 24), dtype=F32)
    for jp, (ay, ax) in enumerate(PAIRS):
        bi[:, jp] = math.log(NORM_COLOR * g7[ay + 3, ax + 3])

    on = np.ones((1, XW), dtype=F16)

    in_maps = []
    for i in range(N_CORES):
        sh = Ip[:, rows * i:rows * i + np_, :]           # [C, np_, PW]
        imt = np.ascontiguousarray(sh.transpose(1, 0, 2))  # [np_, C, PW]
        in_maps.append({"IM": imt, "WT": wt, "BI": bi, "ON": on})
    return in_maps, rows


def assemble(results, him, wim, rows):
    out = np.empty((1, C, him, wim), dtype=F32)
    for i in range(N_CORES):
        out[0, :, rows * i:rows * i + rows, :] = results[i]["OUT"]
    return out


def _numpy_fallback(I, g):
    n, c, h, w = I.shape
    Ipad = np.zeros((n, c, h + 2 * PAD, w + 2 * PAD), dtype=np.float64)
    Ipad[:, :, PAD:PAD + h, PAD:PAD + w] = I
    num = np.zeros((n, c, h, w), dtype=np.float64)
    den = np.zeros((n, h, w), dtype=np.float64)
    g64 = g.astype(np.float64)
    for j in range(K * K):
        dy, dx = j // K, j % K
        S = Ipad[:, :, dy:dy + h, dx:dx + w]
        D = ((S - I.astype(np.float64)) ** 2).sum(axis=1)
        wgt = np.exp(EXP_SCALE * D) * NORM_COLOR * g64[:, j]
        num += wgt[:, None] * S
        den += wgt
    return (num / den[:, None]).astype(F32)


_CACHE = {}
TRACE = False
LAST_EXEC_NS = None
_LDW_PATCHED = False


def _enable_ldw_prune():
    """Drop duplicate LDWEIGHTS of the same stationary lhsT (PE weights
    persist across matmuls)."""
    global _LDW_PATCHED
    if _LDW_PATCHED:
        return
    import json as _json
    import concourse.bass_utils as _bu

    _orig = _bu.compile_bir_kernel

    def _prune(bir_json):
        js = _json.loads(bir_json)
        for fn in js.get("functions", []):
            for blk in fn.get("blocks", []):
                insts = blk.get("instructions", [])
                out = []
                last_ldw = None
                for inst in insts:
                    if inst.get("opcode") == "Ldweights":
                        si = inst.get("sync_info") or {}
                        key = _json.dumps(inst.get("ins"), sort_keys=True)
                        if (
                            last_ldw == key
                            and not si.get("on_wait")
                            and not si.get("on_update")
                        ):
                            continue
                        last_ldw = key
                    out.append(inst)
                blk["instructions"] = out
        return _json.dumps(js).encode()

    def _patched(bir_json, tmpdir, neff_name="file.neff"):
        try:
            bir_json = _prune(bir_json)
        except Exception:
            pass
        return _orig(bir_json, tmpdir, neff_name=neff_name)

    _bu.compile_bir_kernel = _patched
    try:
        import concourse.bass2jax as _b2j

        if getattr(_b2j, "compile_bir_kernel", None) is not None:
            _b2j.compile_bir_kernel = _patched
    except Exception:
        pass
    _LDW_PATCHED = True


def kernel(I: np.ndarray, g: np.ndarray) -> np.ndarray:
    global LAST_EXEC_NS
    I = np.asarray(I, dtype=F32)
    g = np.asarray(g)

    gw49 = np.asarray(g[0, :, 0, 0], dtype=F32)
    g7 = gw49.reshape(K, K)
    spatially_const = np.array_equal(
        np.asarray(g), np.broadcast_to(np.asarray(g)[:, :, :1, :1], g.shape)
    )
    symmetric = np.allclose(g7, g7[::-1, ::-1], rtol=1e-6, atol=0)
    if not (spatially_const and symmetric):
        return _numpy_fallback(I, g)

    from concourse.bass_utils import run_bass_kernel_spmd

    import os as _os
    if _os.environ.get("BASS_LDW_PRUNE", "1") == "1":
        _enable_ldw_prune()

    in_maps, rows = host_prepare(I, gw49)
    key = rows
    if key not in _CACHE:
        _CACHE[key] = build_nc(rows)
    nc = _CACHE[key]
    res = run_bass_kernel_spmd(
        nc, in_maps, core_ids=list(range(N_CORES)), trace=TRACE
    )
    LAST_EXEC_NS = res.exec_time_ns
    return assemble(res.results, I.shape[2], I.shape[3], rows)


if __name__ == "__main__":
    # single-core CoreSim numeric check vs numpy on the core-0 slice
    import concourse.bass_interp as bass_interp

    rng = np.random.default_rng(0)
    I = rng.random((1, C, H, W), dtype=F32)
    gw49 = np.exp(
        -(np.add.outer(np.arange(-3.0, 4) ** 2, np.arange(-3.0, 4) ** 2)) / 50.0
    ).reshape(-1) * (2 * math.pi * 25.0)
    g = np.tile(gw49.reshape(1, K * K, 1, 1), (1, 1, H, W)).astype(F32)

    in_maps, rows = host_prepare(I, gw49.astype(F32))
    nc = build_nc(rows)
    sim = bass_interp.CoreSim(nc)
    for k, v in in_maps[0].items():
        sim.tensor(k)[:] = v
    sim.simulate()
    got = np.array(sim.tensor("OUT"))

    exp_full = _numpy_fallback(I, g)
    exp0 = exp_full[0, :, 0:rows, :]
    err = np.abs(got - exp0)
    print("sim err max:", err.max(), "rel:", err.max() / np.abs(exp0).max())


# revision 5
# speedup vs baseline: 1.6916x; 1.6916x over previous
"""Bilateral filter (7x7, sigma_color=0.1) Trainium2 Bass kernel — Design S2.

Strategy (vs. the strip-layout baseline):
  - Shard H across 8 cores (90 rows each), full 1280 width. Flat row layout:
    SBUF tile IM[102, 3, 1292] fp16 = rows -6..95, channel-major, cols -6..1285.
  - Weight-field symmetry: W_{dy,dx}[u] == W_{6-dy,6-dx}[u + (dy-3,dx-3)], so
    only 24 shift-pairs (+ the free center shift) need D/exp. Per pair j=(ay,ax):
      SB = IM[shifted] - IM[center]            (DVE, [96,3,646] fp16, 2x mode)
      SQ = SB^2                                (ACT Square)
      D  = SQ0+SQ1+SQ2                         (Pool, 2 fused scalar_tensor_tensor)
      F  = exp(-50*D + b_j)                    (ACT, b_j = ln(norm*g_j))
      H  = F * IM[shifted]   (num term j)      (DVE, [90,3,640])
      G  = F * IM[center]    (num term j')     (DVE 2ch + Pool 1ch, [96,3,646])
  - Accumulation on PE into PSUM[90, 4, 640] fp32 (num0..2, den), 2 x-passes
    of 640 cols (PSUM capacity). All matmuls use ONE stationary identity lhsT;
    row/col mirror shifts are rhs partition-offset / column-offset views:
      num += H                      (lhsT=Id, rhs=H)
      num += G[r-ay, x-ax]          (lhsT=Id, rhs=G partitions 3-ay.., cols -ax)
      den += F[r,x] + F[r-ay,x-ax]  (same, rhs=F views)
      center: num += c0*IM, den += c0   (c0 = norm*g_33; tiny extra matmuls)
  - Finalize per pass: rec = 1/den (DVE), out = num*rec (DVE fp32), DMA out.
  - HBM traffic ~2.2 MB/core (vs 38.7 MB for the host-expanded strip design).
"""

import math

import numpy as np

import concourse.bass as bass
import concourse.bacc as bacc
import concourse.mybir as mybir
from concourse.tile import TileContext

F16 = np.float16
F32 = np.float32

H, W, C = 720, 1280, 3
K = 7
PAD = 3
SIGMA_COLOR = 0.1
NORM_COLOR = 1.0 / (2.0 * math.pi * SIGMA_COLOR**2)
EXP_SCALE = -1.0 / (2.0 * SIGMA_COLOR**2)  # -50.0

N_CORES = 8
RPC = H // N_CORES           # 90 output rows per core
XW = 640                     # pass width (PSUM capacity: 4 * 640 fp32 = 5 banks)
NPASS = W // XW              # 2
EXT = 12                     # input halo rows (2*2*PAD)
PW = W + 12                  # padded width 1292

# shift pairs: (ay, ax) with ay<0, or ay==0 and ax<0  (24 of 49; center free)
PAIRS = [(dy - 3, dx - 3) for dy in range(K) for dx in range(K)
         if (dy < 3) or (dy == 3 and dx < 3)]
assert len(PAIRS) == 24

# psum bank-aligned chunks per quantity: (col_start, ncols) within [0, 640),
# global col = 640*q + cs must not cross a 512-col (2KB) bank boundary
CHUNKS = {
    0: [(0, 512), (512, 128)],
    1: [(0, 384), (384, 256)],
    2: [(0, 256), (256, 384)],
    3: [(0, 128), (128, 512)],
}


def _alu(name):
    return getattr(mybir.AluOpType, name)


def build_nc(rows: int = RPC):
    """One core: rows output rows (+12 halo), full width, 2 passes of XW."""
    dt = mybir.dt
    nc = bacc.Bacc("TRN2", debug=False)

    np_ = rows + EXT            # input tile partitions (102)
    fe = rows + 2 * PAD         # F/G extension rows (96)

    IM = nc.dram_tensor("IM", [np_, C, PW], dt.float16, kind="ExternalInput")
    WT = nc.dram_tensor("WT", [fe, 540 + fe], dt.float16, kind="ExternalInput")
    BI = nc.dram_tensor("BI", [128, 24], dt.float32, kind="ExternalInput")
    ON = nc.dram_tensor("ON", [1, XW], dt.float16, kind="ExternalInput")
    OUT = nc.dram_tensor("OUT", [C, rows, W], dt.float32, kind="ExternalOutput")

    with TileContext(nc) as tc:
        with (
            tc.tile_pool(name="persist", bufs=1) as pp,
            tc.tile_pool(name="sub", bufs=3) as psub,
            tc.tile_pool(name="sq", bufs=3) as psq,
            tc.tile_pool(name="ff", bufs=4) as pf,
            tc.tile_pool(name="hh", bufs=3) as ph,
            tc.tile_pool(name="gg", bufs=3) as pg,
            tc.tile_pool(name="fin", bufs=2) as pfin,
            tc.tile_pool(name="psum", bufs=1, space="PSUM") as pps,
            tc.tile_pool(name="psumd", bufs=1, space="PSUM") as ppsd,
        ):
            # SBUF compute ops require base partition 0 (or 32/64/96), so
            # row shifts can't be partition-offset views. Instead build 7
            # row-shifted SBUF copies via DMA (no partition restriction):
            # IMS[d][p, c, xi] = Ipad[row p-3+d, col xi],  p=0..95
            IMS = {}
            for d in range(-PAD, PAD + 1):
                t = pp.tile([fe, C, PW], dt.float16, tag=f"ims{d}")
                nc.sync.dma_start(t[:, :, :], IM[3 + d:3 + d + fe, :, :])
                IMS[d] = t
            wt = pp.tile([fe, 540 + fe], dt.float16, tag="wt")
            nc.sync.dma_start(wt[:, :], WT[:, :])
            bi = pp.tile([128, 24], dt.float32, tag="bi")
            nc.sync.dma_start(bi[:, :], BI[:, :])
            ones = pp.tile([1, XW], dt.float16, tag="ones")
            nc.sync.dma_start(ones[:, :], ON[:, :])

            # lhsT views, all base partition 0:
            # SH[k][p, r] = 1 iff p == r + k (k=3..6); CID = c0*SH[3]; OC = c0 row
            SH = {k: wt[:, (k - 3) * rows:(k - 2) * rows] for k in range(3, 7)}
            CID = wt[:, 4 * rows:5 * rows]
            OC = wt[0:1, 5 * rows:6 * rows]
            I96 = wt[:, 6 * rows:6 * rows + fe]   # identity [fe, fe]

            for p_i in range(NPASS):
                x0 = p_i * XW
                ps = pps.tile([rows, 4, XW], dt.float32, tag="ps", name=f"ps{p_i}")
                started = set()  # PSUM 2KB banks with an open accumulation group

                def mm(q, lhsT, rhs_of, stop_banks=None):
                    """Accumulate psum[:, q, :] += lhsT.T @ rhs chunks.
                    start=True only on the first matmul touching a bank
                    (marks the whole bank pending-zero); stop closes it."""
                    for ci, (cs, cn) in enumerate(CHUNKS[q]):
                        bank = (2560 * q + 4 * cs) // 2048
                        st = bank not in started
                        started.add(bank)
                        sp = stop_banks is not None and bank in stop_banks
                        nc.tensor.matmul(
                            ps[:, q, cs:cs + cn], lhsT, rhs_of(cs, cn),
                            start=st, stop=sp,
                        )

                for jp, (ay, ax) in enumerate(PAIRS):
                    # --- D / F over extension region: partition p = r'+3,
                    #     r' in -3..rows+2; tile col xi = x'-x0+3, x' in -3..642
                    sb = psub.tile([fe, C, 646], dt.float16, tag="sb", name="sb")
                    nc.vector.tensor_tensor(
                        sb[:, :, :],
                        IMS[ay][:, :, x0 + 3 + ax:x0 + 649 + ax],
                        IMS[0][:, :, x0 + 3:x0 + 649],
                        _alu("subtract"),
                    )
                    sq = psq.tile([fe, C, 646], dt.float16, tag="sq", name="sq")
                    nc.scalar.activation(
                        sq[:, :, :], sb[:, :, :],
                        mybir.ActivationFunctionType.Square,
                    )
                    # channel sum on PE: D-psum += sq_c via identity matmuls
                    # (GpSimd contends with DVE for SBUF; PE does this free)
                    psd = ppsd.tile([fe, 1024], dt.float32, tag="psd", name="psd")
                    for c in range(C):
                        for cs, cn in ((0, 512), (512, 134)):
                            nc.tensor.matmul(
                                psd[:, cs:cs + cn], I96,
                                sq[:, c, cs:cs + cn],
                                start=(c == 0), stop=(c == C - 1),
                            )
                    ff = pf.tile([fe, 646], dt.float16, tag="ff", name="ff")
                    nc.scalar.activation(
                        ff[:, :], psd[:, 0:646],
                        mybir.ActivationFunctionType.Exp,
                        bias=bi[0:fe, jp:jp + 1], scale=float(EXP_SCALE),
                    )
                    # --- products
                    hh = ph.tile([fe, C, XW], dt.float16, tag="hh", name="hh")
                    nc.vector.tensor_tensor(
                        hh[:, :, :],
                        ff[:, None, 3:3 + XW].to_broadcast([fe, C, XW]),
                        IMS[ay][:, :, x0 + 6 + ax:x0 + 646 + ax],
                        _alu("mult"),
                    )
                    gg = pg.tile([fe, C, 646], dt.float16, tag="gg", name="gg")
                    nc.vector.tensor_tensor(
                        gg[:, :, :],
                        ff[:, None, :].to_broadcast([fe, C, 646]),
                        IMS[0][:, :, x0 + 3:x0 + 649],
                        _alu("mult"),
                    )
                    # --- accumulate: H-term (SH[3]) and G-term (SH[3-ay],
                    # col shift via rhs view)
                    ks = 3 - ay
                    for q in range(C):
                        mm(q, SH[3], lambda cs, cn, q=q: hh[:, q, cs:cs + cn])
                    mm(3, SH[3], lambda cs, cn: ff[:, 3 + cs:3 + cs + cn])
                    for q in range(C):
                        mm(q, SH[ks], lambda cs, cn, q=q: gg[
                            :, q, 3 - ax + cs:3 - ax + cs + cn])
                    mm(3, SH[ks], lambda cs, cn: ff[
                        :, 3 - ax + cs:3 - ax + cs + cn])

                # --- center shift: num += c0*I, den += c0. Last matmul
                # touching each bank closes its accumulation group.
                cen = [(q, cs, cn) for q in range(C) for cs, cn in CHUNKS[q]]
                cen += [(3, cs, cn) for cs, cn in CHUNKS[3]]
                banks = [(2560 * q + 4 * cs) // 2048 for q, cs, cn in cen]
                for i, (q, cs, cn) in enumerate(cen):
                    rhs = (ones[:, cs:cs + cn] if q == 3
                           else IMS[0][:, q, x0 + 6 + cs:x0 + 6 + cs + cn])
                    nc.tensor.matmul(
                        ps[:, q, cs:cs + cn], OC if q == 3 else CID, rhs,
                        start=False, stop=banks[i] not in banks[i + 1:],
                    )

                # --- finalize pass (den is well inside fp32 normal range)
                rec = pfin.tile([rows, XW], dt.float32, tag="rec", name="rec")
                nc.vector.reciprocal_approx_fast(rec[:, :], ps[:, 3, :])
                ot = pfin.tile([rows, C, XW], dt.float32, tag="ot", name="ot")
                nc.vector.tensor_tensor(
                    ot[:, :, :], ps[:, 0:3, :],
                    rec[:, None, :].to_broadcast([rows, C, XW]),
                    _alu("mult"),
                )
                for c in range(C):
                    nc.sync.dma_start(OUT[c, :, x0:x0 + XW], ot[:, c, :])

    nc.compile()
    return nc


def host_prepare(I: np.ndarray, gw49: np.ndarray):
    """I: (1, C, H, W) fp32, gw49: (49,). Returns per-core input maps."""
    _, c_, him, wim = I.shape
    rows = him // N_CORES
    np_ = rows + EXT

    Ip = np.zeros((C, him + EXT, wim + EXT), dtype=F16)
    Ip[:, 6:6 + him, 6:6 + wim] = I[0].astype(F16)

    g7 = gw49.reshape(K, K).astype(np.float64)
    c0 = float(NORM_COLOR * g7[3, 3])

    fe = rows + 2 * PAD
    wt = np.zeros((fe, 540 + fe), dtype=F16)
    idx = np.arange(rows)
    for k in range(3, 7):                  # SH[k]: p == r+k
        wt[idx + k, (k - 3) * rows + idx] = 1.0
    wt[idx + 3, 4 * rows + idx] = c0       # CID = c0 * SH[3]
    wt[0, 5 * rows:6 * rows] = c0          # OC row
    ide = np.arange(fe)
    wt[ide, 6 * rows + ide] = 1.0          # I96 identity

    bi = np.zeros((128, 24), dtype=F32)
    for jp, (ay, ax) in enumerate(PAIRS):
        bi[:, jp] = math.log(NORM_COLOR * g7[ay + 3, ax + 3])

    on = np.ones((1, XW), dtype=F16)

    in_maps = []
    for i in range(N_CORES):
        sh = Ip[:, rows * i:rows * i + np_, :]           # [C, np_, PW]
        imt = np.ascontiguousarray(sh.transpose(1, 0, 2))  # [np_, C, PW]
        in_maps.append({"IM": imt, "WT": wt, "BI": bi, "ON": on})
    return in_maps, rows


def assemble(results, him, wim, rows):
    out = np.empty((1, C, him, wim), dtype=F32)
    for i in range(N_CORES):
        out[0, :, rows * i:rows * i + rows, :] = results[i]["OUT"]
    return out


def _numpy_fallback(I, g):
    n, c, h, w = I.shape
    Ipad = np.zeros((n, c, h + 2 * PAD, w + 2 * PAD), dtype=np.float64)
    Ipad[:, :, PAD:PAD + h, PAD:PAD + w] = I
    num = np.zeros((n, c, h, w), dtype=np.float64)
    den = np.zeros((n, h, w), dtype=np.float64)
    g64 = g.astype(np.float64)
    for j in range(K * K):
        dy, dx = j // K, j % K
        S = Ipad[:, :, dy:dy + h, dx:dx + w]
        D = ((S - I.astype(np.float64)) ** 2).sum(axis=1)
        wgt = np.exp(EXP_SCALE * D) * NORM_COLOR * g64[:, j]
        num += wgt[:, None] * S
        den += wgt
    return (num / den[:, None]).astype(F32)


_CACHE = {}
TRACE = False
LAST_EXEC_NS = None
_LDW_PATCHED = False


def _enable_ldw_prune():
    """Drop duplicate LDWEIGHTS of the same stationary lhsT (PE weights
    persist across matmuls)."""
    global _LDW_PATCHED
    if _LDW_PATCHED:
        return
    import json as _json
    import concourse.bass_utils as _bu

    _orig = _bu.compile_bir_kernel

    def _prune(bir_json):
        js = _json.loads(bir_json)
        for fn in js.get("functions", []):
            for blk in fn.get("blocks", []):
                insts = blk.get("instructions", [])
                out = []
                last_ldw = None
                for inst in insts:
                    if inst.get("opcode") == "Ldweights":
                        si = inst.get("sync_info") or {}
                        key = _json.dumps(inst.get("ins"), sort_keys=True)
                        if (
                            last_ldw == key
                            and not si.get("on_wait")
                            and not si.get("on_update")
                        ):
                            continue
                        last_ldw = key
                    out.append(inst)
                blk["instructions"] = out
        return _json.dumps(js).encode()

    def _patched(bir_json, tmpdir, neff_name="file.neff"):
        try:
            bir_json = _prune(bir_json)
        except Exception:
            pass
        return _orig(bir_json, tmpdir, neff_name=neff_name)

    _bu.compile_bir_kernel = _patched
    try:
        import concourse.bass2jax as _b2j

        if getattr(_b2j, "compile_bir_kernel", None) is not None:
            _b2j.compile_bir_kernel = _patched
    except Exception:
        pass
    _LDW_PATCHED = True


def kernel(I: np.ndarray, g: np.ndarray) -> np.ndarray:
    global LAST_EXEC_NS
    I = np.asarray(I, dtype=F32)
    g = np.asarray(g)

    gw49 = np.asarray(g[0, :, 0, 0], dtype=F32)
    g7 = gw49.reshape(K, K)
    spatially_const = np.array_equal(
        np.asarray(g), np.broadcast_to(np.asarray(g)[:, :, :1, :1], g.shape)
    )
    symmetric = np.allclose(g7, g7[::-1, ::-1], rtol=1e-6, atol=0)
    if not (spatially_const and symmetric):
        return _numpy_fallback(I, g)

    from concourse.bass_utils import run_bass_kernel_spmd

    import os as _os
    if _os.environ.get("BASS_LDW_PRUNE", "1") == "1":
        _enable_ldw_prune()

    in_maps, rows = host_prepare(I, gw49)
    key = rows
    if key not in _CACHE:
        _CACHE[key] = build_nc(rows)
    nc = _CACHE[key]
    res = run_bass_kernel_spmd(
        nc, in_maps, core_ids=list(range(N_CORES)), trace=TRACE
    )
    LAST_EXEC_NS = res.exec_time_ns
    return assemble(res.results, I.shape[2], I.shape[3], rows)


if __name__ == "__main__":
    # single-core CoreSim numeric check vs numpy on the core-0 slice
    import concourse.bass_interp as bass_interp

    rng = np.random.default_rng(0)
    I = rng.random((1, C, H, W), dtype=F32)
    gw49 = np.exp(
        -(np.add.outer(np.arange(-3.0, 4) ** 2, np.arange(-3.0, 4) ** 2)) / 50.0
    ).reshape(-1) * (2 * math.pi * 25.0)
    g = np.tile(gw49.reshape(1, K * K, 1, 1), (1, 1, H, W)).astype(F32)

    in_maps, rows = host_prepare(I, gw49.astype(F32))
    nc = build_nc(rows)
    sim = bass_interp.CoreSim(nc)
    for k, v in in_maps[0].items():
        sim.tensor(k)[:] = v
    sim.simulate()
    got = np.array(sim.tensor("OUT"))

    exp_full = _numpy_fallback(I, g)
    exp0 = exp_full[0, :, 0:rows, :]
    err = np.abs(got - exp0)
    print("sim err max:", err.max(), "rel:", err.max() / np.abs(exp0).max())
